# revision 36
# baseline (speedup 1.0000x reference)
"""Trainium2 Bass kernel for an attention block (B=8, T=2048, D=K=V=1024).

Reference math (per batch element, sharded one per NeuronCore):
    Q = x @ Wq.T + bq ; K = x @ Wk.T + bk ; V = x @ Wv.T + bv
    logits[t,s] = Q[t] . K[s],  masked -inf for s > t (strict upper tri)
    probs = softmax(logits, axis=t) / sqrt(1024)     # softmax over QUERY axis
    out = x + probs @ V

v7: all matmuls fp8 (e4m3) DoubleRow.  Two-region schedule:
  REGION 1 (projections): W row-tiles stream DMA -> x32 bf16 -> PE
    transpose (4 per PSUM tile, 1 drain) as in v4; x block 0 likewise on
    the PE, x blocks 1-3 via batched xbar DMA transposes ([128,2048] ->
    [128,16,128]) emitted a block ahead of use.  Q/K/V for each t-block
    run back-to-back per block; xT8 is transient (bufs=2) since block j
    is only read while projecting block j.  Q lands in 4 persistent
    k-major fp8 blocks, K in KT8, V in Vp8.
  REGION 2 (dense sweep, column-major over s): for each s-tile sv, the
    4 causal logits tiles (j = sv//4..3) -> exp (Z via accum_out) into
    transient tiles; Z -> R = 1/Z; normalize into Pq8 fp8; PV out-tile
    sv-2 lags so the exp->R chain stays off the PE critical path.  This
    removes all phase-boundary stalls and makes Pbig transient.
  - Diagonal logits tiles (j == sv//4) narrowed to columns >= 128*(sv%4);
    the dead Pq8 blocks that PV's DoubleRow round-up reads are memset.
  - Residual epilogue: out = psum*(1/1024) + x via one vector
    scalar_tensor_tensor per half-tile (no id1k matmuls).
Measured numerics: rel_err ~4.6e-3 (tolerance 2e-2).
"""

import time

import numpy as np

import concourse.bass as bass
import concourse.bacc as bacc
import concourse.mybir as mybir
import concourse.tile as tile
from concourse.bass_utils import run_bass_kernel_spmd
from concourse.masks import make_identity

F32 = mybir.dt.float32
BF16 = mybir.dt.bfloat16
FP8 = mybir.dt.float8e4
AF = mybir.ActivationFunctionType
DR = mybir.MatmulPerfMode.DoubleRow
MULT = mybir.AluOpType.mult
ADD = mybir.AluOpType.add

P = 128          # partitions
T = 2048         # sequence length
D = 1024         # model dim
TB = 512         # t-block width
NTB = 4          # t-blocks
KO = 8           # k output tiles of 128
DK = 8           # contraction subtiles of 128
SV = 16          # s tiles of 128
NEG = -1.0e30
WS = 32.0        # weight quantization scale


def _build_nc():
    nc = bacc.Bacc("TRN2", target_bir_lowering=False, debug=False, num_devices=8)

    x = nc.dram_tensor("x", [T, D], F32, kind="ExternalInput").ap()
    Wq = nc.dram_tensor("Wq", [D, D], F32, kind="ExternalInput").ap()
    bq = nc.dram_tensor("bq", [D], F32, kind="ExternalInput").ap()
    Wk = nc.dram_tensor("Wk", [D, D], F32, kind="ExternalInput").ap()
    bk = nc.dram_tensor("bk", [D], F32, kind="ExternalInput").ap()
    Wv = nc.dram_tensor("Wv", [D, D], F32, kind="ExternalInput").ap()
    bv = nc.dram_tensor("bv", [D], F32, kind="ExternalInput").ap()
    out = nc.dram_tensor("out", [T, D], F32, kind="ExternalOutput").ap()

    with tile.TileContext(nc) as tc:
        _kernel_body(nc, tc, x, Wq, bq, Wk, bk, Wv, bv, out)

    nc.compile()
    return nc


def _kernel_body(nc, tc, x, Wq, bq, Wk, bk, Wv, bv, out):
    from contextlib import ExitStack

    ctx = ExitStack()
    with ctx:
        consts = ctx.enter_context(tc.tile_pool(name="consts", bufs=1))
        wt8p = ctx.enter_context(tc.tile_pool(name="wt8", bufs=1))
        xt8p = ctx.enter_context(tc.tile_pool(name="xt8", bufs=2))
        kt8p = ctx.enter_context(tc.tile_pool(name="kt8", bufs=1))
        vp8p = ctx.enter_context(tc.tile_pool(name="vp8", bufs=1))
        pq8p = ctx.enter_context(tc.tile_pool(name="pq8", bufs=1))
        qt8p = ctx.enter_context(tc.tile_pool(name="qt8", bufs=1))
        xbp = ctx.enter_context(tc.tile_pool(name="xb", bufs=1))
        pexpp = ctx.enter_context(tc.tile_pool(name="pexp", bufs=6))
        natp = ctx.enter_context(tc.tile_pool(name="nat", bufs=8))
        wbp = ctx.enter_context(tc.tile_pool(name="wb", bufs=3))
        xtp = ctx.enter_context(tc.tile_pool(name="xt", bufs=2))
        ostp = ctx.enter_context(tc.tile_pool(name="ost", bufs=3))
        # one shared 8-bank PSUM ring for transposes and matmuls: deep
        # buffering at the front (transpose drains) and in the sweep
        # (logits psums held until exp no longer starve PV psums)
        psum_mm = ctx.enter_context(tc.tile_pool(name="psum_mm", bufs=8, space="PSUM"))

        # identity gates the PE transposes at kernel start
        id32 = consts.tile([P, P], F32, name="id32")
        make_identity(nc, id32)
        idb = consts.tile([P, P], BF16, name="idb")
        nc.vector.tensor_copy(out=idb, in_=id32)

        # persistent fp8 operand tensors
        WqT8 = wt8p.tile([P, DK, D], FP8, name="WqT8")   # (32 Wq)^T [d_in, dk, k]
        WkT8 = wt8p.tile([P, DK, D], FP8, name="WkT8")
        WvT8 = wt8p.tile([P, DK, D], FP8, name="WvT8")
        xT8b = [xt8p.tile([P, DK, TB], FP8, name="xT8", tag="xT8")
                for _ in range(NTB)]                     # x^T per t-block
        KT8 = kt8p.tile([P, KO, T], FP8, name="KT8")     # (K+bk)^T [k_in, ko, s]
        Vp8 = vp8p.tile([P, SV, D], FP8, name="Vp8")     # 32(V+bv) [s_in, sv, v]
        QT8b = [qt8p.tile([P, KO, TB], FP8, name=f"QT8_{j}")
                for j in range(NTB)]                     # Q^T per t-block
        Pq8 = [pq8p.tile([P, 4 * j + 4, TB], FP8, name=f"Pq8_{j}")
               for j in range(NTB)]                      # P/Z [s_in, sv, t] per j
        xb = xbp.tile([P, SV, D], BF16, name="xb")       # x rows bf16 (residual)

        # dead Pq8 blocks read by the PV DoubleRow round-up but never
        # written once the diagonal logits tiles are narrowed
        for j in range(NTB):
            nc.vector.memset(Pq8[j][:, 4 * j + 1, 0:P], 0.0)
            nc.vector.memset(Pq8[j][:, 4 * j + 3, 2 * P:3 * P], 0.0)

        Zacc = consts.tile([P, SV, NTB], F32, name="Zacc")
        nc.vector.memset(Zacc, 0.0)
        ztmp = consts.tile([P, SV], F32, name="ztmp")
        rtile = consts.tile([P, SV], F32, name="rtile")

        # ---- front DMAs: x tiles 0-3 split gpsimd+sync; Wq 0-3 gpsimd ----
        def dma_in_split(dst, src, nsplit=2):
            step = P // nsplit
            for q in range(nsplit):
                eng = nc.gpsimd if q % 2 == 0 else nc.sync
                eng.dma_start(out=dst[q * step:(q + 1) * step, :],
                              in_=src[q * step:(q + 1) * step, :])

        def dma_gp(dst, src):
            nc.gpsimd.dma_start(out=dst, in_=src)

        xnat_pre = []
        for ti in range(4):
            xnat = natp.tile([P, D], F32, name="xnat", tag="nat")
            dma_in_split(xnat, x[ti * P:(ti + 1) * P, :], nsplit=4)
            xnat_pre.append(xnat)
        wq_pre = []
        for kt in range(4):
            wnat = natp.tile([P, D], F32, name="wnat", tag="nat")
            dma_in_split(wnat, Wq[kt * P:(kt + 1) * P, :], nsplit=2)
            wq_pre.append(wnat)

        # biases after the x halves on sync (needed only at first epilogue)
        bq_sb = consts.tile([P, KO], F32, name="bq_sb")
        nc.sync.dma_start(out=bq_sb, in_=bq.rearrange("(o p) -> p o", p=P))
        bk_sb = consts.tile([P, KO], F32, name="bk_sb")
        nc.sync.dma_start(out=bk_sb, in_=bk.rearrange("(o p) -> p o", p=P))
        bv_sb = consts.tile([P, D], F32, name="bv_sb")
        bv_bcast = bass.AP(tensor=bv.tensor, offset=bv.offset,
                           ap=[[0, P], [1, D]])
        nc.sync.dma_start(out=bv_sb, in_=bv_bcast)

        # mask build on gpsimd compute
        mask_base = consts.tile([P, TB + 3 * P], BF16, name="mask_base")
        nc.gpsimd.memset(mask_base, 0.0)
        nc.gpsimd.affine_select(
            out=mask_base, in_=mask_base,
            compare_op=mybir.AluOpType.is_ge,
            fill=NEG,
            base=-(3 * P),
            pattern=[[1, TB + 3 * P]],
            channel_multiplier=-1,
        )
        masks = [mask_base[:, 3 * P - oi * P: 3 * P - oi * P + TB]
                 for oi in range(4)]

        bv32_sb = consts.tile([P, D], BF16, name="bv32_sb")

        eng_ctr = [0]

        def alt_copy(dst, src, scale=None):
            """Copy/scale-copy alternating between vector and scalar."""
            eng_ctr[0] += 1
            if eng_ctr[0] % 2 == 0:
                if scale is None:
                    nc.vector.tensor_copy(out=dst, in_=src)
                else:
                    nc.vector.tensor_scalar_mul(dst, src, scale)
            else:
                nc.scalar.activation(dst, src, AF.Copy,
                                     scale=1.0 if scale is None else scale)

        grp_ctr = [0]

        def transpose_group(srcb, dst, dk0, dst_col):
            """4 bf16 PE transposes into one [128,512] psum tile, 1 drain."""
            pt = psum_mm.tile([P, 4 * P], BF16, name="pt", tag="mm")
            for q in range(4):
                dk = dk0 + q
                nc.tensor.transpose(
                    pt[:, q * P:(q + 1) * P],
                    srcb[:, dk * P:(dk + 1) * P], idb)
            dview = dst[:, dk0:dk0 + 4, dst_col:dst_col + P]
            pview = pt.rearrange("p (g c) -> p g c", g=4)
            if grp_ctr[0] % 2 == 0:
                nc.vector.tensor_copy(out=dview, in_=pview)
            else:
                nc.scalar.activation(dview, pview, AF.Copy)
            grp_ctr[0] += 1

        def emit_x_tile_pe(ti, xnat=None):
            """x tile via PE transpose: self-paced feed work on the PE."""
            if xnat is None:
                xnat = natp.tile([P, D], F32, name="xnat", tag="nat")
                dma_in_split(xnat, x[ti * P:(ti + 1) * P, :], nsplit=2)
            alt_copy(xb[:, ti, :], xnat)
            for dk0 in (0, 4):
                transpose_group(xb[:, ti, :], xT8b[ti // 4], dk0, (ti % 4) * P)

        def emit_x_pair_xbar(ti):
            """x tiles ti, ti+1: DMA f32 -> xb bf16 -> one xbar -> xT8 fp8."""
            j = ti // 4
            for u in range(2):
                xnat = natp.tile([P, D], F32, name="xnat", tag="nat")
                dma_gp(xnat, x[(ti + u) * P:(ti + u + 1) * P, :])
                alt_copy(xb[:, ti + u, :], xnat)
            xt = xtp.tile([P, 16, P], BF16, name="xt", tag="xt")
            nc.sync.dma_start(out=xt, in_=xb[:, ti:ti + 2, :], transpose=True)
            c0 = (ti % 4) * P
            for u in range(2):
                alt_copy(xT8b[j][:, :, c0 + u * P:c0 + (u + 1) * P],
                         xt[:, 8 * u:8 * u + 8, :])

        def emit_w_tile(w_ap, dst, kt, wnat=None):
            """W row-tile kt: (DMA'd) f32 -> x32 bf16 (in halves, so the
            first transpose group starts half a copy earlier) -> PE."""
            if wnat is None:
                wnat = natp.tile([P, D], F32, name="wnat", tag="nat")
                dma_in_split(wnat, w_ap[kt * P:(kt + 1) * P, :], nsplit=2)
            wb = wbp.tile([P, D], BF16, name="wb", tag="wb")
            alt_copy(wb, wnat, scale=WS)
            for dk0 in (0, 4):
                transpose_group(wb, dst, dk0, kt * P)

        def emit_qkt_ko(j, wt8, bias_sb, dst, ko):
            """One QT/KT column tile [k 128, t 512], fused epilogue -> fp8."""
            ps = psum_mm.tile([P, TB], F32, name="ps_qk", tag="mm")
            for a in range(4):
                nc.tensor.matmul(
                    ps,
                    lhsT=wt8[:, 2 * a:2 * a + 2, ko * P:(ko + 1) * P],
                    rhs=xT8b[j][:, 2 * a:2 * a + 2, :],
                    start=(a == 0), stop=(a == 3),
                    perf_mode=DR,
                )
            if ko % 2 == 0:
                nc.vector.tensor_scalar(
                    out=dst, in0=ps,
                    scalar1=1.0 / WS, scalar2=bias_sb[:, ko:ko + 1],
                    op0=MULT, op1=ADD,
                )
            else:
                nc.scalar.activation(
                    dst, ps, AF.Identity,
                    bias=bias_sb[:, ko:ko + 1], scale=1.0 / WS,
                )

        def emit_v_unit(j, si, h):
            """One Vp8 tile [s 128, v 512] = 32(V+bv) fp8."""
            sv = 4 * j + si
            ps = psum_mm.tile([P, TB], F32, name="ps_v", tag="mm")
            for a in range(4):
                nc.tensor.matmul(
                    ps,
                    lhsT=xT8b[j][:, 2 * a:2 * a + 2, si * P:(si + 1) * P],
                    rhs=WvT8[:, 2 * a:2 * a + 2, h * TB:(h + 1) * TB],
                    start=(a == 0), stop=(a == 3),
                    perf_mode=DR,
                )
            nc.vector.tensor_add(
                out=Vp8[:, sv, h * TB:(h + 1) * TB],
                in0=ps, in1=bv32_sb[:, h * TB:(h + 1) * TB],
            )

        def emit_logits_exp(j, sv):
            """logits tile [s 128, t 512-col0] -> exp (Z accum) -> pexp.

            Only j == sv//4 can be diagonal; it skips the fully-masked
            columns t < 128*(sv%4)."""
            oi = sv - 4 * j
            col0 = P * oi if oi > 0 else 0
            width = TB - col0
            ps = psum_mm.tile([P, TB], F32, name="ps_l", tag="mm")
            for a in range(4):
                nc.tensor.matmul(
                    ps[:, 0:width],
                    lhsT=KT8[:, 2 * a:2 * a + 2, sv * P:(sv + 1) * P],
                    rhs=QT8b[j][:, 2 * a:2 * a + 2, col0:TB],
                    start=(a == 0), stop=(a == 3),
                    perf_mode=DR,
                )
            if oi >= 0:
                nc.vector.tensor_add(out=ps[:, 0:width], in0=ps[:, 0:width],
                                     in1=masks[oi][:, col0:TB])
            pexp = pexpp.tile([P, TB], BF16, name="pexp", tag="pexp")
            nc.scalar.activation(
                pexp[:, 0:width], ps[:, 0:width], AF.Exp,
                accum_out=Zacc[:, sv, j:j + 1],
            )
            return pexp

        def emit_out_tile(i):
            """out rows [i*128, (i+1)*128): PV fp8 DR; fused epilogue
            out = psum/1024 + x on the vector engine."""
            jj = i // 4
            tc_ = i % 4
            npair = (i + 2) // 2
            for h in range(D // TB):
                ps = psum_mm.tile([P, TB], F32, name="ps_o", tag="mm")
                for a in range(npair):
                    nc.tensor.matmul(
                        ps,
                        lhsT=Pq8[jj][:, 2 * a:2 * a + 2, tc_ * P:(tc_ + 1) * P],
                        rhs=Vp8[:, 2 * a:2 * a + 2, h * TB:(h + 1) * TB],
                        start=(a == 0), stop=(a == npair - 1),
                        perf_mode=DR,
                    )
                oh = ostp.tile([P, TB], F32, name="oh", tag="ost")
                nc.vector.scalar_tensor_tensor(
                    out=oh, in0=ps, scalar=1.0 / (WS * WS),
                    in1=xb[:, i, h * TB:(h + 1) * TB],
                    op0=MULT, op1=ADD,
                )
                nc.sync.dma_start(
                    out=out[i * P:(i + 1) * P, h * TB:(h + 1) * TB], in_=oh)

        # ---- REGION 1: projections ----
        # x block 0 on the PE, interleaved with early Wq DMAs
        for ti in range(4):
            emit_x_tile_pe(ti, xnat_pre[ti])
        nc.scalar.activation(bv32_sb, bv_sb, AF.Copy, scale=WS)

        # unified W stream, software-pipelined: the chain for tile i+LAG
        # is emitted before the consumer matmuls of tile i, so section
        # transitions (Wq->Wk->Wv) never drain the DMA/copy/transpose
        # pipeline ahead of the PE.
        def emit_q(j, ko):
            emit_qkt_ko(j, WqT8, bq_sb, QT8b[j][:, ko, :], ko)

        def emit_k(j, ko):
            emit_qkt_ko(j, WkT8, bk_sb, KT8[:, ko, j * TB:(j + 1) * TB], ko)

        def w_chain(i):
            sec, kt = divmod(i, 8)
            if sec == 0:
                if kt < 4:
                    wnat = natp.tile([P, D], F32, name="wnat", tag="nat")
                    dma_in_split(wnat, Wq[(kt + 4) * P:(kt + 5) * P, :],
                                 nsplit=2)
                    wq_pre.append(wnat)
                emit_w_tile(Wq, WqT8, kt, wnat=wq_pre[kt])
            elif sec == 1:
                emit_w_tile(Wk, WkT8, kt)
            else:
                emit_w_tile(Wv, WvT8, kt)

        def w_consume(i):
            """Wv chains also feed the x block 1 PE-transpose chains so
            the PE stays dense through that stretch."""
            sec, kt = divmod(i, 8)
            if sec == 0:
                emit_q(0, kt)
            elif sec == 1:
                emit_k(0, kt)
            else:
                if kt < 4:
                    emit_x_tile_pe(4 + kt)   # block 1
                if kt == 3:
                    for si in range(4):
                        emit_v_unit(0, si, 0)
                if kt == 7:
                    for si in range(4):
                        emit_v_unit(0, si, 1)

        LAG = 2
        for i in range(24 + LAG):
            if i < 24:
                w_chain(i)
            if i >= LAG:
                w_consume(i - LAG)

        for j in range(1, NTB):
            if j < NTB - 1:
                for ti in range(4):
                    emit_x_tile_pe(4 * (j + 1) + ti)  # block j+1
            for ko in range(KO):
                emit_q(j, ko)
            for ko in range(KO):
                emit_k(j, ko)
            for si in range(TB // P):
                for h in range(D // TB):
                    emit_v_unit(j, si, h)

        # ---- REGION 2: dense column-major softmax/PV sweep ----
        for sv in range(SV):
            j0 = sv // 4
            pexps = {}
            for j in range(j0, NTB):
                pexps[j] = emit_logits_exp(j, sv)
            nc.vector.reduce_sum(out=ztmp[:, sv:sv + 1],
                                 in_=Zacc[:, sv, :],
                                 axis=mybir.AxisListType.X)
            nc.vector.reciprocal(rtile[:, sv:sv + 1], ztmp[:, sv:sv + 1])
            for jp in range(j0, NTB):
                oi2 = sv - 4 * jp
                col0 = P * oi2 if oi2 > 0 else 0
                src = pexps[jp][:, 0:TB - col0]
                dstq = Pq8[jp][:, sv, col0:TB]
                if (jp + sv) % 2 == 0:
                    nc.vector.tensor_scalar_mul(
                        dstq, src, rtile[:, sv:sv + 1])
                else:
                    nc.scalar.activation(
                        dstq, src, AF.Identity, scale=rtile[:, sv:sv + 1])
            if sv >= 2:
                emit_out_tile(sv - 2)
        emit_out_tile(SV - 2)
        emit_out_tile(SV - 1)


_NC_CACHE = None


def _get_nc():
    global _NC_CACHE
    if _NC_CACHE is None:
        _NC_CACHE = _build_nc()
    return _NC_CACHE


def kernel(minibatch, Wq, bq, Wk, bk, Wv, bv):
    minibatch = np.asarray(minibatch, dtype=np.float32)
    Wq = np.asarray(Wq, dtype=np.float32)
    bq = np.asarray(bq, dtype=np.float32)
    Wk = np.asarray(Wk, dtype=np.float32)
    bk = np.asarray(bk, dtype=np.float32)
    Wv = np.asarray(Wv, dtype=np.float32)
    bv = np.asarray(bv, dtype=np.float32)

    nc = _get_nc()
    B = minibatch.shape[0]
    in_maps = [
        {
            "x": np.ascontiguousarray(minibatch[i]),
            "Wq": Wq, "bq": bq, "Wk": Wk, "bk": bk, "Wv": Wv, "bv": bv,
        }
        for i in range(B)
    ]
    last_err = None
    for _attempt in range(3):
        try:
            res = run_bass_kernel_spmd(nc, in_maps, core_ids=list(range(B)))
            break
        except Exception as e:  # transient device errors
            last_err = e
            time.sleep(2.0)
    else:
        raise last_err
    return np.stack([res.results[i]["out"] for i in range(B)], axis=0)
